# revision 23
# baseline (speedup 1.0000x reference)
"""Multi-head attention with RoPE on 8 Trainium2 NeuronCores.

Sharding: core c handles batch b = c//2 and head-group hg = c%2 (8 of 16
heads).  Data-parallel over batch, tensor-parallel over heads; the
row-parallel wo all-reduce (2 cores per batch) happens on the host during
the gather/unshard step.

v2 per-core program (single SPMD NEFF, no collectives):
  - bf16 storage for x^T, weights, Q/K/V, attention weights and output
    (psum accumulation stays f32); halves DMA and SBUF traffic.
  - Projections: QT/KT = w @ x^T with 8-step psum accumulation; V in
    [s, 8*(64+1)] layout with a ones column per head (softmax denominator
    rides the attn@V matmul for free).
  - RoPE: host permutes wq/wk rows per head so the pair-swap is a
    16<->16 swap inside each 32-partition quadrant -> one DVE
    stream_shuffle (no SBUF<->SBUF DMA), then mul/mul/add with
    precomputed cos/sin tables (DVE + gpsimd).
  - Attention per (head-pair, q-chunk): for each 128-k-block j, the two
    heads' scores^T go into one 2-bank psum tile [128, 1024] via two
    row-tiled (64-contraction) matmuls that run concurrently on the PE;
    one exp ACTIVATE covers both heads (halves ACT instruction
    overhead); causal tri-mask on gpsimd; attn@V accumulates [65, 512]
    per head in psum.  Normalize via reciprocal (DVE) + ones-outer
    broadcast matmul (PE) + multiply (DVE).
  - wo projection per q-chunk is interleaved right after each q-chunk's
    attention so output DMA overlaps the next chunk's compute.
"""

import sys
import types

sys.path.insert(0, "/opt/trn_rl_repo")

import numpy as np

import concourse.bacc as bacc
import concourse.mybir as mybir
import concourse.tile as tile
from concourse.bass_utils import run_bass_kernel_spmd

# Problem constants (hardcoded per contract)
B, S, D = 4, 2048, 1024
H = 16
DH = D // H          # 64
THETA = 10000.0
NCORES = 8
HG = 2               # head groups (tensor-parallel factor)
HD = D // HG         # 512 = per-core heads dim
NH = H // HG         # 8 heads per core
P = 128
SC = 512             # q-chunk
NSC = S // SC        # 4
NKB = S // P         # 16 k-blocks
NDB = D // P         # 8 d-blocks (contraction for projections)
SCALE = 1.0 / np.sqrt(np.float32(DH))

F32 = mybir.dt.float32
F32R = mybir.dt.float32r
BF16 = mybir.dt.bfloat16

# stream_shuffle: swap halves within each 32-partition quadrant
SHUF_MASK = list(range(16, 32)) + list(range(16))


def _install_ntff_hook():
    """Best-effort: register the axon NTFF profile hook so trace=True works."""
    try:
        import antenv

        if "antenv.axon_hooks" in sys.modules:
            return
        sys.path.insert(0, "/root/.axon_site/trn_agent_boot")
        import trn_boot

        hook = trn_boot._ntff_profile_via_ctypes("/opt/axon/libaxon_pjrt.so")
        mod = types.ModuleType("antenv.axon_hooks")
        mod.get_axon_ntff_profile_hook = lambda: hook
        mod.set_axon_ntff_profile_hook = lambda h: None
        sys.modules["antenv.axon_hooks"] = mod
        antenv.axon_hooks = mod
    except Exception:
        pass


def build_program():
    nc = bacc.Bacc("TRN2", target_bir_lowering=False, debug=False,
                   num_devices=NCORES)

    xt_d = nc.dram_tensor("xt", [D, S], BF16, kind="ExternalInput")
    wqt_d = nc.dram_tensor("wqt", [D, HD], BF16, kind="ExternalInput")
    wkt_d = nc.dram_tensor("wkt", [D, HD], BF16, kind="ExternalInput")
    wvt_d = nc.dram_tensor("wvt", [D, HD], BF16, kind="ExternalInput")
    wot_d = nc.dram_tensor("wot", [HD, D], BF16, kind="ExternalInput")
    cf_d = nc.dram_tensor("cfull", [P, S], BF16, kind="ExternalInput")
    sf_d = nc.dram_tensor("sfull", [P, S], BF16, kind="ExternalInput")
    tri_d = nc.dram_tensor("tri", [P, P], BF16, kind="ExternalInput")
    onesr_d = nc.dram_tensor("onesr", [1, DH], F32R, kind="ExternalInput")
    onesb_d = nc.dram_tensor("onesb", [P, NH], BF16, kind="ExternalInput")
    negc_d = nc.dram_tensor("negc", [P, 3 * P], BF16, kind="ExternalInput")
    out_d = nc.dram_tensor("outT", [D, S], BF16, kind="ExternalOutput")

    EXP = mybir.ActivationFunctionType.Exp
    MULT = mybir.AluOpType.mult
    ADD = mybir.AluOpType.add

    with tile.TileContext(nc) as tc:
        with (
            tc.tile_pool(name="xtp", bufs=NDB) as xtp,
            tc.tile_pool(name="qk", bufs=8) as qk,
            tc.tile_pool(name="vg", bufs=NKB) as vgp,
            tc.tile_pool(name="w", bufs=24) as wp,
            tc.tile_pool(name="wo", bufs=4) as wop,
            tc.tile_pool(name="ap", bufs=4) as ap_pool,
            tc.tile_pool(name="at2", bufs=5) as at2p,
            tc.tile_pool(name="sw", bufs=2) as swp,
            tc.tile_pool(name="small", bufs=8) as small,
            tc.tile_pool(name="ot", bufs=4) as otp,
            tc.tile_pool(name="psA", bufs=2, space="PSUM") as psA,
            tc.tile_pool(name="psS", bufs=2, space="PSUM") as psS,
            tc.tile_pool(name="psV", bufs=1, space="PSUM") as psV,
        ):
            # ---- constants / DMAs (weights first so Q-proj starts early) ----
            ones64 = small.tile([1, DH], F32R, tag="ones64", bufs=1)
            nc.sync.dma_start(ones64[:], onesr_d[:])
            ones_sb = small.tile([P, NH], BF16, tag="onesb", bufs=1)
            nc.sync.dma_start(ones_sb[:], onesb_d[:])
            negt = small.tile([P, 3 * P], BF16, tag="negc", bufs=1)
            nc.sync.dma_start(negt[:], negc_d[:])

            wq_t = []
            for k in range(NDB):
                t = wp.tile([P, HD], BF16, tag="w", name=f"wq{k}")
                nc.sync.dma_start(t[:], wqt_d[P * k:P * (k + 1), :])
                wq_t.append(t)
            xt = []
            for k in range(NDB):
                t = xtp.tile([P, S], BF16, tag="xt", name=f"xt{k}")
                nc.sync.dma_start(t[:], xt_d[P * k:P * (k + 1), :])
                xt.append(t)
            wk_t = []
            for k in range(NDB):
                t = wp.tile([P, HD], BF16, tag="w", name=f"wk{k}")
                nc.sync.dma_start(t[:], wkt_d[P * k:P * (k + 1), :])
                wk_t.append(t)
            wv_t = []
            for k in range(NDB):
                t = wp.tile([P, HD], BF16, tag="w", name=f"wv{k}")
                nc.sync.dma_start(t[:], wvt_d[P * k:P * (k + 1), :])
                wv_t.append(t)
            cf = small.tile([P, S], BF16, tag="cf", bufs=1)
            nc.sync.dma_start(cf[:], cf_d[:])
            sf = small.tile([P, S], BF16, tag="sf", bufs=1)
            nc.sync.dma_start(sf[:], sf_d[:])
            tri = small.tile([P, P], BF16, tag="tri", bufs=1)
            nc.sync.dma_start(tri[:], tri_d[:])
            wo_t = []
            for k in range(HD // P):
                t = wop.tile([P, D], BF16, tag="wot", name=f"wo{k}")
                nc.sync.dma_start(t[:], wot_d[P * k:P * (k + 1), :])
                wo_t.append(t)

            # ---- Q/K projections + RoPE, one q-chunk column slice at a time
            # (lets chunk qc+1's projection PE work overlap chunk qc's
            # ACT-heavy attention) ----
            def rope_slice(t, n):
                cs = slice(SC * n, SC * (n + 1))
                sw = swp.tile([P, SC], BF16, tag="sw", name="sw")
                nc.vector.stream_shuffle(sw[:], t[:, cs], SHUF_MASK)
                nc.gpsimd.tensor_tensor(sw[:], sw[:], sf[:, cs], MULT)
                nc.gpsimd.tensor_tensor(t[:, cs], t[:, cs], cf[:, cs], MULT)
                nc.vector.tensor_tensor(t[:, cs], t[:, cs], sw[:], ADD)

            def project_slice(wt, out_tiles, n):
                for m in range(HD // P):
                    ps = psA.tile([P, SC], F32, tag="psA")
                    for k in range(NDB):
                        nc.tensor.matmul(
                            ps[:],
                            (wt[k][:, P * m:P * (m + 1)]),
                            (xt[k][:, SC * n:SC * (n + 1)]),
                            start=(k == 0), stop=(k == NDB - 1),
                        )
                    nc.vector.tensor_copy(
                        out_tiles[m][:, SC * n:SC * (n + 1)], ps[:])
                    rope_slice(out_tiles[m], n)

            QT = [qk.tile([P, S], BF16, tag=f"qt{m}", bufs=1, name=f"qt{m}")
                  for m in range(HD // P)]
            KT = [qk.tile([P, S], BF16, tag=f"kt{m}", bufs=1, name=f"kt{m}")
                  for m in range(HD // P)]
            project_slice(wq_t, QT, 0)
            project_slice(wk_t, KT, 0)

            # ---- V projection (j = k-block); interleaved with attention ----
            Vg = [None] * NKB

            def vproj(j):
                vt = vgp.tile([P, NH * (DH + 1)], BF16, tag=f"vg{j}", bufs=1,
                              name=f"vg{j}")
                v3 = vt[:].rearrange("p (h c) -> p h c", h=NH)
                ps = psA.tile([P, HD], F32, tag="psA")
                for k in range(NDB):
                    nc.tensor.matmul(
                        ps[:],
                        (xt[k][:, P * j:P * (j + 1)]),
                        (wv_t[k][:]),
                        start=(k == 0), stop=(k == NDB - 1),
                    )
                nc.vector.tensor_copy(
                    v3[:, :, 0:DH], ps[:].rearrange("p (h c) -> p h c", h=NH))
                nc.vector.tensor_copy(v3[:, :, DH:DH + 1], ones_sb[:, :, None])
                Vg[j] = vt

            for j in range(4):
                vproj(j)

            # ---- attention for one (q-chunk, head-pair) ----
            A = [ap_pool.tile([P, S], BF16, tag=f"a{m}", bufs=1, name=f"a{m}")
                 for m in range(HD // P)]

            def attn(qc, hp):
                h0, h1 = 2 * hp, 2 * hp + 1
                nj = 4 * qc + 4
                av = psV.tile([DH + 1, 2 * SC], F32, tag="psV", name="av")
                ps_l = [None] * nj
                at_l = [None] * nj

                def scores(j):
                    d = j - 4 * qc
                    q0 = P * d if d >= 0 else 0
                    ps = psS.tile([P, 2 * SC], F32, tag="psS", name="ps")
                    if q0 > 0:
                        nc.vector.tensor_copy(ps[:, SC:SC + q0], negt[:, 0:q0])
                    nc.tensor.matmul(
                        ps[:, q0:SC],
                        (KT[hp][0:DH, P * j:P * (j + 1)]),
                        (QT[hp][0:DH, SC * qc + q0:SC * (qc + 1)]),
                        start=True, stop=True,
                    )
                    nc.tensor.matmul(
                        ps[:, SC + q0:2 * SC],
                        (KT[hp][DH:P, P * j:P * (j + 1)]),
                        (QT[hp][DH:P, SC * qc + q0:SC * (qc + 1)]),
                        start=True, stop=True,
                    )
                    at2 = at2p.tile([P, 2 * SC], BF16, tag="at2", name="at2")
                    nc.scalar.activation(at2[:, q0:2 * SC], ps[:, q0:2 * SC],
                                         EXP, scale=float(SCALE))
                    ps_l[j] = ps
                    at_l[j] = at2

                def do_av(j):
                    d = j - 4 * qc
                    q0 = P * d if d >= 0 else 0
                    at2 = at_l[j]
                    if d >= 0:
                        at3 = at2[:].rearrange("p (g c) -> p g c", g=2)
                        nc.gpsimd.tensor_tensor(
                            at3[:, 0, q0:q0 + P], at3[:, 0, q0:q0 + P],
                            tri[:], MULT)
                        nc.gpsimd.tensor_tensor(
                            at3[:, 1, q0:q0 + P], at3[:, 1, q0:q0 + P],
                            tri[:], MULT)
                    nc.tensor.matmul(
                        av[:, q0:SC],
                        (Vg[j][:, (DH + 1) * h0:(DH + 1) * (h0 + 1)]),
                        (at2[:, q0:SC]),
                        start=(j == 0), stop=(j == nj - 1),
                    )
                    nc.tensor.matmul(
                        av[:, SC + q0:2 * SC],
                        (Vg[j][:, (DH + 1) * h1:(DH + 1) * (h1 + 1)]),
                        (at2[:, SC + q0:2 * SC]),
                        start=(j == 0), stop=(j == nj - 1),
                    )
                    at_l[j] = None
                    ps_l[j] = None

                for j in range(nj):
                    scores(j)
                    if j > 1:
                        do_av(j - 2)
                do_av(nj - 2)
                do_av(nj - 1)

                # 1/denom = exp(-log(denom)) on ACT (2-ULP tables), both
                # heads in one pass; copy av out of psum early so the 2
                # banks free fast.
                LOG = mybir.ActivationFunctionType.Ln
                avr = small.tile([DH, 2 * SC], BF16, tag="avr", bufs=2,
                                 name="avr")
                nc.vector.tensor_copy(avr[:], av[0:DH, :])
                ld = small.tile([1, 2 * SC], F32, tag="ld", bufs=2, name="ld")
                nc.scalar.activation(ld[:], av[DH:DH + 1, :], LOG)
                rr = small.tile([1, 2 * SC], F32R, tag="rr", bufs=2, name="rr")
                with nc.allow_low_precision(reason="f32r matmul feed"):
                    nc.scalar.activation(rr[:], ld[:], EXP, scale=-1.0)
                for hh in (0, 1):
                    bc = psA.tile([DH, SC], F32, tag="psA", name="bc")
                    nc.tensor.matmul(bc[:], (ones64[:]),
                                     (rr[:, SC * hh:SC * (hh + 1)]),
                                     start=True, stop=True)
                    rb = small.tile([DH, SC], F32, tag="rb", bufs=4, name="rb")
                    nc.vector.tensor_copy(rb[:], bc[:])
                    nc.vector.tensor_tensor(
                        A[hp][DH * hh:DH * (hh + 1), SC * qc:SC * (qc + 1)],
                        avr[:, SC * hh:SC * (hh + 1)], rb[:], MULT)

            # ---- output projection for one q-chunk ----
            def woproj(qc):
                for m in range(D // P):
                    ps = psA.tile([P, SC], F32, tag="psA")
                    for k in range(HD // P):
                        nc.tensor.matmul(
                            ps[:],
                            (wo_t[k][:, P * m:P * (m + 1)]),
                            (A[k][:, SC * qc:SC * (qc + 1)]),
                            start=(k == 0), stop=(k == HD // P - 1),
                        )
                    ot = otp.tile([P, SC], BF16, tag="ot")
                    nc.vector.tensor_copy(ot[:], ps[:])
                    nc.sync.dma_start(
                        out_d[P * m:P * (m + 1), SC * qc:SC * (qc + 1)], ot[:])

            # ---- main schedule ----
            for qc in range(NSC):
                for hp in range(HD // P):
                    attn(qc, hp)
                    if qc < NSC - 1:
                        if hp == 0:
                            project_slice(wq_t, QT, qc + 1)
                        elif hp == 1:
                            project_slice(wk_t, KT, qc + 1)
                        elif hp == 2:
                            for j in range(4 * qc + 4, 4 * qc + 8):
                                vproj(j)
                woproj(qc)

    nc.compile()
    return nc


_NC_CACHE = []


def _get_nc():
    if not _NC_CACHE:
        _NC_CACHE.append(build_program())
    return _NC_CACHE[0]


def _host_tables(token_positions):
    """cos/sin tables [128, S] matching the 16|16 quadrant row layout."""
    pos = np.asarray(token_positions).astype(np.float32)
    inv_freq = np.float32(THETA) ** (
        -np.arange(0, DH, 2, dtype=np.float32) / np.float32(DH))  # [32]
    ang = pos[:, None] * inv_freq[None, :]                # [S, 32]
    cos_t = np.cos(ang).T.astype(np.float32)              # [32, S]
    sin_t = np.sin(ang).T.astype(np.float32)
    # quadrant q (of 4): freqs 16*(q%2) .. +16, rows [c|c] / [-s|+s]
    crows, srows = [], []
    for q in range(4):
        f = slice(16 * (q % 2), 16 * (q % 2) + 16)
        crows += [cos_t[f], cos_t[f]]
        srows += [-sin_t[f], sin_t[f]]
    return np.concatenate(crows, 0), np.concatenate(srows, 0)


def _perm():
    """Per-head-pair row permutation: 16 even dims | 16 odd dims per
    32-row quadrant (so the RoPE pair-swap is intra-quadrant)."""
    perm1 = []
    for q in range(2):  # two quadrants per head
        perm1 += [2 * (16 * q + i) for i in range(16)]
        perm1 += [2 * (16 * q + i) + 1 for i in range(16)]
    perm1 = np.array(perm1)
    return np.concatenate([h * DH + perm1 for h in range(NH)])


def build_in_maps(in_features, token_positions, wq, wk, wv, wo):
    x = np.asarray(in_features, dtype=np.float32)
    wq = np.asarray(wq, dtype=np.float32)
    wk = np.asarray(wk, dtype=np.float32)
    wv = np.asarray(wv, dtype=np.float32)
    wo = np.asarray(wo, dtype=np.float32)

    cfull, sfull = _host_tables(token_positions)
    tri = np.triu(np.ones((P, P), dtype=np.float32))   # keep k_row <= q_col
    perm = _perm()
    bf = np.dtype("bfloat16") if hasattr(np, "bfloat16") else None

    def b16(a):
        import ml_dtypes
        return np.ascontiguousarray(a).astype(ml_dtypes.bfloat16)

    in_maps = []
    for c in range(NCORES):
        b, hg = divmod(c, HG)
        sl = slice(hg * HD, (hg + 1) * HD)
        in_maps.append({
            "xt": b16(x[b].T),
            "wqt": b16(wq[sl][perm].T),
            "wkt": b16(wk[sl][perm].T),
            "wvt": b16(wv[sl].T),
            "wot": b16(wo[:, sl].T),
            "cfull": b16(cfull),
            "sfull": b16(sfull),
            "tri": b16(tri),
            "onesr": np.ones((1, DH), dtype=np.float32),
            "onesb": b16(np.ones((P, NH), dtype=np.float32)),
            "negc": b16(np.full((P, 3 * P), -1e30, dtype=np.float32)),
        })
    return in_maps


def kernel(in_features, token_positions, wq, wk, wv, wo):
    _install_ntff_hook()
    in_maps = build_in_maps(in_features, token_positions, wq, wk, wv, wo)
    nc = _get_nc()
    res = run_bass_kernel_spmd(nc, in_maps, list(range(NCORES)))

    out = np.empty((B, S, D), dtype=np.float32)
    for b in range(B):
        acc = (np.asarray(res.results[2 * b]["outT"]).astype(np.float32)
               + np.asarray(res.results[2 * b + 1]["outT"]).astype(np.float32))
        out[b] = acc.T
    return out
